# revision 18
# baseline (speedup 1.0000x reference)
"""MDTA-style dense attention (B=2, N=4096+8 summary tokens, C=192, H=8, D=24)
on 8 Trainium2 NeuronCores.

Sharding: data-parallel over batch B (2) x tensor-parallel over heads
(4 groups of 2 heads) -> 8 cores. Each core computes attention for one batch
and two heads plus its slice of the qkv projection and the output projection
partial sum; partials are summed on the host.

Device algorithm per core (feature-major layouts):
  - One merged qk projection matmul emits q (temperature-folded) and
    k' = (128/ln2)*k for both heads; V_aug in [keys, d] layout via per-block
    matmuls (ones-indicator feature row produces the denominator column and
    zeros for padded keys).
  - Head h streams on PE row-strips {2h, 2h+1}. Each strip carries a 25th
    bias row (q side 1.0, k side 16256.0) so the score matmul lands
    z = (128*log2 e)*S + 16256 in PSUM directly.
  - "exp": alternating per group between ScalarE ACTIVATE(Exp) (true exp,
    scaled by 1.0407 to match the trick's mean) and a DVE fp32->int16 cast
    whose int16 bits are the bf16 encoding of 2^(z/128-127) (Schraudolph);
    softmax normalization cancels the common factor.
  - PV accumulates both heads into one PSUM bank (h0's first matmul covers
    the full 64 columns via a zero-padded weight tile so the bank-wide
    has_written clear stays correct).
  - Denominator rows sit adjacently (strip cols 31/32); one
    reciprocal_approx_fast + two gpsimd partition broadcasts + one fused
    multiply produce normalized head outputs.
  - Output projection is transposed: otn chunks stationary, Wout streamed,
    giving out[q, c] tiles DMAd straight to a [N, C] output.
"""

import numpy as np

import concourse.bass as bass
import concourse.tile as tile
from concourse import bacc, mybir
from concourse.bass_utils import run_bass_kernel_spmd

# Problem constants (hardcoded per contract).
B = 2
N = 4096          # output tokens
K_SUM = 8         # summary tokens
NT = N + K_SUM    # 4104 total tokens
NP = 4224         # padded key count = 33 * 128
C = 192
H = 8
D = 24
NCORES = 8

CI = 512          # query chunk (8 chunks over 4096)
MB = 128          # key block
GROUP = 2         # key blocks per exp group
NCHUNKS = N // CI            # 8
MBLOCKS = NP // MB           # 33
NGROUPS = (MBLOCKS + GROUP - 1) // GROUP  # 17
NSLOTS = (MBLOCKS + 1) // 2  # 17 k-block column slots per strip

LOG2E_128 = 128.0 / float(np.log(2.0))   # fold into Wk on host
ZBIAS = 16256.0                          # 127*128, exact in fp16
# ScalarE true-exp: out = exp(scale*z + bias); the exp(ln 1.040684) factor
# matches the Schraudolph trick's mean multiplicative bias so mixed groups
# stay consistent (softmax cancels the common factor).
ACT_SCALE = float(np.log(2.0)) / 128.0
ACT_BIAS = float(-ZBIAS * np.log(2.0) / 128.0 + np.log(1.040684))

F32 = mybir.dt.float32
F16 = mybir.dt.float16
BF16 = mybir.dt.bfloat16
I16 = mybir.dt.int16

_CACHED = {}


def build_program(num_devices=NCORES):
    nc = bacc.Bacc("TRN2", target_bir_lowering=False, debug=False,
                   num_devices=num_devices)
    xt_d = nc.dram_tensor("XT", [C + 1, NP], F16, kind="ExternalInput")
    wt_d = nc.dram_tensor("WT", [C + 1, 160], F16, kind="ExternalInput")
    wo_d = nc.dram_tensor("WoT", [64, C], F16, kind="ExternalInput")
    out_d = nc.dram_tensor("outT", [N, C], F16, kind="ExternalOutput")

    with tile.TileContext(nc) as tc:
        with tc.tile_pool(name="singles", bufs=1) as singles:
            xt0 = singles.tile([128, NP], F16, tag="xt0")
            xt1 = singles.tile([65, NP], F16, tag="xt1")
            wt0 = singles.tile([128, 160], F16, tag="wt0")
            wt1 = singles.tile([65, 160], F16, tag="wt1")
            wo = singles.tile([64, C], F16, tag="wo")
            qks = singles.tile([96, NP], F16, tag="qks")
            q4 = singles.tile([128, N], F16, tag="q4")
            k4 = singles.tile([128, NSLOTS * MB], F16, tag="k4")
            # Per-head zero-padded V_aug weight tiles: every PV matmul writes
            # the full 64-partition output (tile_position (0,0)) so no
            # col-group-offset matmuls are needed and the first matmul's
            # bank-wide has_written clear covers both heads.
            vaugp = [singles.tile([128, MBLOCKS, 64], BF16, tag=f"vaugp{h}",
                                  name=f"vaugp{h}") for h in range(2)]
            otn = singles.tile([64, N], F16, tag="otn")
            actb = singles.tile([128, 1], F32, tag="actb")
            nc.vector.memset(actb[:], ACT_BIAS)

            # Bias rows for the score matmuls (25th row of each strip):
            # q side streams 1.0, k side holds 16256.0 so PSUM z includes
            # the exponent bias. Padded key columns get z=16256 -> e~1,
            # but their V_aug rows are all zero, contributing nothing.
            # Bias rows live at strip row 24; memsets must start 32-aligned,
            # so fill whole strips and let the q/k scatter DMAs overwrite
            # rows 0:24 (rows 25:31 keep the value but are never streamed).
            for s in range(4):
                nc.vector.memset(q4[32 * s:32 * s + 32, :], 1.0)
                nc.vector.memset(k4[32 * s:32 * s + 32, :], ZBIAS)
            nc.vector.memset(vaugp[0][:, :, :], 0.0)
            nc.vector.memset(vaugp[1][:, :, :], 0.0)

            # Input loads; weights first so the first matmuls aren't queued
            # behind the large XT transfer. XT chunked to start compute early.
            nc.sync.dma_start(out=wt0[:], in_=wt_d[0:128, :])
            nc.sync.dma_start(out=wt1[:], in_=wt_d[128:193, :])
            nc.sync.dma_start(out=wo[:], in_=wo_d[:, :])
            for c0 in range(0, NP, 1056):
                nc.sync.dma_start(out=xt0[:, c0:c0 + 1056],
                                  in_=xt_d[0:128, c0:c0 + 1056])
                nc.gpsimd.dma_start(out=xt1[:, c0:c0 + 1056],
                                    in_=xt_d[128:193, c0:c0 + 1056])

            xts = (xt0, xt1)
            wts = (wt0, wt1)

            # ---- merged qk projection: out rows [q_h0|q_h1|k_h0|k_h1] ----
            with tc.tile_pool(name="qkp", bufs=3, space="PSUM") as qkp:
                for ci in range(9):
                    c0 = ci * CI
                    w = CI if ci < 8 else NP - 8 * CI
                    ps = qkp.tile([96, CI], F32, tag="qk")
                    for kc in range(2):
                        nc.tensor.matmul(
                            ps[:, :w],
                            lhsT=wts[kc][:, 0:96],
                            rhs=xts[kc][:, c0:c0 + w],
                            start=(kc == 0), stop=(kc == 1))
                    if ci % 2 == 0:
                        nc.scalar.copy(out=qks[:, c0:c0 + w], in_=ps[:, :w])
                    else:
                        nc.vector.tensor_copy(out=qks[:, c0:c0 + w],
                                              in_=ps[:, :w])

            # ---- V_aug production: [keys, d] layout ----
            with tc.tile_pool(name="vps", bufs=4, space="PSUM") as vps:
                for mb in range(MBLOCKS):
                    m0 = mb * MB
                    ps = vps.tile([128, 64], F32, tag="v")
                    for kc in range(2):
                        nc.tensor.matmul(
                            ps[:],
                            lhsT=xts[kc][:, m0:m0 + MB],
                            rhs=wts[kc][:, 96:160],
                            start=(kc == 0), stop=(kc == 1))
                    if mb % 2 == 0:
                        nc.scalar.copy(out=vaugp[0][:, mb, 0:32],
                                       in_=ps[:, 0:32])
                        nc.vector.tensor_copy(out=vaugp[1][:, mb, 32:64],
                                              in_=ps[:, 32:64])
                    else:
                        nc.vector.tensor_copy(out=vaugp[0][:, mb, 0:32],
                                              in_=ps[:, 0:32])
                        nc.scalar.copy(out=vaugp[1][:, mb, 32:64],
                                       in_=ps[:, 32:64])

            # ---- scatter q into head strips, k blocks into strip slots ----
            # strip s (partitions 32s..32s+24) serves head s//2; k block j of
            # head h lands on strip 2h+(j&1), column slot j>>1.
            for s in range(4):
                h = s // 2
                for half in range(2):
                    cq = half * (N // 2)
                    eng = nc.sync if (s + half) % 2 == 0 else nc.gpsimd
                    eng.dma_start(
                        out=q4[32 * s:32 * s + 24, cq:cq + N // 2],
                        in_=qks[24 * h:24 * h + 24, cq:cq + N // 2])
            for j in range(MBLOCKS):
                for h in range(2):
                    s = 2 * h + (j & 1)
                    slot = j >> 1
                    eng = nc.sync if (j + h) % 2 == 0 else nc.gpsimd
                    eng.dma_start(
                        out=k4[32 * s:32 * s + 24,
                               slot * MB:(slot + 1) * MB],
                        in_=qks[48 + 24 * h:48 + 24 * h + 24,
                                j * MB:(j + 1) * MB])

            # ---- attention ----
            flats = [(g, h) for g in range(NGROUPS) for h in range(2)]
            with (tc.tile_pool(name="sp", bufs=3, space="PSUM") as sp,
                  tc.tile_pool(name="op", bufs=2, space="PSUM") as op,
                  tc.tile_pool(name="ep", bufs=7) as ep,
                  tc.tile_pool(name="bp", bufs=2) as bp):
                for ci in range(NCHUNKS):
                    c0 = ci * CI
                    o_ps = op.tile([64, CI], F32, tag="o")
                    LAG = 4  # PV trails its exp by LAG flats to keep PE dense
                    pend = {}
                    for i in range(len(flats) + LAG):
                        if i < len(flats):
                            g, h = flats[i]
                            nblk = GROUP if g < NGROUPS - 1 else 1
                            s_ps = sp.tile([128, GROUP, CI], F32, tag="s")
                            for j in range(nblk):
                                mb = GROUP * g + j
                                s = 2 * h + (mb & 1)
                                p0 = 32 * s
                                slot = mb >> 1
                                nc.tensor.matmul(
                                    s_ps[:, j, :],
                                    lhsT=k4[p0:p0 + 25,
                                            slot * MB:(slot + 1) * MB],
                                    rhs=q4[p0:p0 + 25, c0:c0 + CI],
                                    start=True, stop=True,
                                    tile_position=(p0, 0),
                                    skip_group_check=True)
                            e_t = ep.tile([128, GROUP, CI], I16, tag="e")
                            if i % 2 == 0:
                                nc.scalar.activation(
                                    out=e_t[:, 0:nblk, :].bitcast(BF16),
                                    in_=s_ps[:, 0:nblk, :],
                                    func=mybir.ActivationFunctionType.Exp,
                                    bias=actb[:, :], scale=ACT_SCALE)
                            else:
                                nc.vector.tensor_copy(
                                    out=e_t[:, 0:nblk, :],
                                    in_=s_ps[:, 0:nblk, :])
                            pend[i] = e_t
                        k_ = i - LAG
                        if k_ >= 0 and k_ < len(flats):
                            g, h = flats[k_]
                            nblk = GROUP if g < NGROUPS - 1 else 1
                            e_t = pend.pop(k_)
                            for j in range(nblk):
                                mb = GROUP * g + j
                                first = (k_ == 0 and j == 0)
                                nc.tensor.matmul(
                                    o_ps[0:64, :],
                                    lhsT=vaugp[h][:, mb, :],
                                    rhs=e_t[:, j, :].bitcast(BF16),
                                    start=first, stop=(mb == MBLOCKS - 1),
                                    tile_position=(0, 0),
                                    skip_group_check=True)
                    # normalize: denominator rows are strip cols 0 (h0) and
                    # 32 (h1); engine APs need 32-aligned start partitions,
                    # hence two reciprocal ops into separate base-0 tiles.
                    # reciprocal_approx_fast and partition_broadcast only
                    # work on base-0 APs on hardware; DMA moves h1's
                    # denominator row to partition 0 and its broadcast back
                    # to partitions 32:64 (DMAs handle any partitions).
                    dsb = bp.tile([33, CI], F32, tag="dsb")
                    nc.vector.tensor_copy(out=dsb[0:33, :], in_=o_ps[0:33, :])
                    d2 = bp.tile([1, CI], F32, tag="d2")
                    nc.sync.dma_start(out=d2[0:1, :], in_=dsb[32:33, :])
                    rr0 = bp.tile([1, CI], F32, tag="rr0")
                    rr1 = bp.tile([1, CI], F32, tag="rr1")
                    nc.vector.reciprocal_approx_fast(out=rr0[0:1, :],
                                                     in_=dsb[0:1, :])
                    nc.vector.reciprocal_approx_fast(out=rr1[0:1, :],
                                                     in_=d2[0:1, :])
                    bc = bp.tile([32, CI], F32, tag="bc")
                    bc2 = bp.tile([32, CI], F32, tag="bc2")
                    bc3 = bp.tile([64, CI], F32, tag="bc3")
                    nc.gpsimd.partition_broadcast(bc[0:32, :], rr0[0:1, :])
                    nc.gpsimd.partition_broadcast(bc2[0:32, :], rr1[0:1, :])
                    nc.gpsimd.dma_start(out=bc3[32:64, :], in_=bc2[0:32, :])
                    nc.vector.tensor_mul(out=otn[0:32, c0:c0 + CI],
                                         in0=o_ps[0:32, :], in1=bc[:, :])
                    nc.vector.tensor_mul(out=otn[32:64, c0:c0 + CI],
                                         in0=o_ps[32:64, :],
                                         in1=bc3[32:64, :])

            # ---- output projection (transposed: out[q, c] tiles) ----
            with (tc.tile_pool(name="pp", bufs=4, space="PSUM") as pp,
                  tc.tile_pool(name="ob", bufs=4) as ob):
                for t in range(N // 128):
                    q0 = t * 128
                    ps = pp.tile([128, C], F32, tag="p")
                    nc.tensor.matmul(ps[:], lhsT=otn[:, q0:q0 + 128],
                                     rhs=wo[:, :], start=True, stop=True)
                    so = ob.tile([128, C], F16, tag="so")
                    if t % 2 == 0:
                        nc.scalar.copy(out=so[:], in_=ps[:])
                    else:
                        nc.vector.tensor_copy(out=so[:], in_=ps[:])
                    eng = nc.sync if t % 2 == 0 else nc.gpsimd
                    eng.dma_start(out=out_d[q0:q0 + 128, :], in_=so[:])

    nc.compile()
    return nc


def make_in_maps(X_flat, S_tokens, Wqkv, Wout, temperature):
    temp = np.asarray(temperature, dtype=np.float32).reshape(H)
    Wq = np.asarray(Wqkv[0:C], dtype=np.float32)
    Wk = np.asarray(Wqkv[C:2 * C], dtype=np.float32)
    Wv = np.asarray(Wqkv[2 * C:3 * C], dtype=np.float32)
    Wout = np.asarray(Wout, dtype=np.float32)

    xts = []
    for b in range(B):
        x_in = np.concatenate([np.asarray(X_flat[b], dtype=np.float32),
                               np.asarray(S_tokens[b], dtype=np.float32)],
                              axis=0)
        xt = np.zeros((C + 1, NP), dtype=np.float32)
        xt[:C, :NT] = np.ascontiguousarray(x_in.T)
        xt[C, :NT] = 1.0  # indicator -> ones column of V_aug
        xts.append(xt)

    in_maps = []
    for core in range(NCORES):
        b = core // 4
        h0 = 2 * (core % 4)
        h1 = h0 + 1
        wt = np.zeros((C + 1, 160), dtype=np.float32)
        wt[:C, 0:24] = (Wq[h0 * D:(h0 + 1) * D] * temp[h0]).T
        wt[:C, 24:48] = (Wq[h1 * D:(h1 + 1) * D] * temp[h1]).T
        wt[:C, 48:72] = (Wk[h0 * D:(h0 + 1) * D] * LOG2E_128).T
        wt[:C, 72:96] = (Wk[h1 * D:(h1 + 1) * D] * LOG2E_128).T
        # V_aug columns: h0 ones at 0, v at 1:25; h1 ones at 32, v at 33:57
        # (denominator rows land at 32-aligned strip cols 0 and 32).
        wt[C, 96] = 1.0
        wt[:C, 97:121] = Wv[h0 * D:(h0 + 1) * D].T
        wt[C, 128] = 1.0
        wt[:C, 129:153] = Wv[h1 * D:(h1 + 1) * D].T
        wo = np.zeros((64, C), dtype=np.float32)
        wo[1:25] = Wout[:, h0 * D:(h0 + 1) * D].T
        wo[33:57] = Wout[:, h1 * D:(h1 + 1) * D].T
        in_maps.append({
            "XT": np.ascontiguousarray(xts[b]).astype(np.float16),
            "WT": np.ascontiguousarray(wt).astype(np.float16),
            "WoT": np.ascontiguousarray(wo).astype(np.float16),
        })
    return in_maps


def run(in_maps, **kwargs):
    if "nc" not in _CACHED:
        _CACHED["nc"] = build_program()
    return run_bass_kernel_spmd(_CACHED["nc"], in_maps,
                                core_ids=list(range(NCORES)), **kwargs)


def kernel(X_flat, S_tokens, Wqkv, Wout, temperature):
    in_maps = make_in_maps(X_flat, S_tokens, Wqkv, Wout, temperature)
    res = run(in_maps)
    out = np.zeros((B, N, C), dtype=np.float32)
    for core in range(NCORES):
        out[core // 4] += res.results[core]["outT"].astype(np.float32)
    return out


# revision 19
# speedup vs baseline: 1.0296x; 1.0296x over previous
"""MDTA-style dense attention (B=2, N=4096+8 summary tokens, C=192, H=8, D=24)
on 8 Trainium2 NeuronCores.

Sharding: data-parallel over batch B (2) x tensor-parallel over heads
(4 groups of 2 heads) -> 8 cores. Each core computes attention for one batch
and two heads plus its slice of the qkv projection and the output projection
partial sum; partials are summed on the host.

Device algorithm per core (feature-major layouts):
  - One merged qk projection matmul emits q (temperature-folded) and
    k' = (128/ln2)*k for both heads; V_aug in [keys, d] layout via per-block
    matmuls (ones-indicator feature row produces the denominator column and
    zeros for padded keys).
  - Head h streams on PE row-strips {2h, 2h+1}. Each strip carries a 25th
    bias row (q side 1.0, k side 16256.0) so the score matmul lands
    z = (128*log2 e)*S + 16256 in PSUM directly.
  - "exp": alternating per group between ScalarE ACTIVATE(Exp) (true exp,
    scaled by 1.0407 to match the trick's mean) and a DVE fp32->int16 cast
    whose int16 bits are the bf16 encoding of 2^(z/128-127) (Schraudolph);
    softmax normalization cancels the common factor.
  - PV accumulates both heads into one PSUM bank (h0's first matmul covers
    the full 64 columns via a zero-padded weight tile so the bank-wide
    has_written clear stays correct).
  - Denominator rows sit adjacently (strip cols 31/32); one
    reciprocal_approx_fast + two gpsimd partition broadcasts + one fused
    multiply produce normalized head outputs.
  - Output projection is transposed: otn chunks stationary, Wout streamed,
    giving out[q, c] tiles DMAd straight to a [N, C] output.
"""

import numpy as np

import concourse.bass as bass
import concourse.tile as tile
from concourse import bacc, mybir
from concourse.bass_utils import run_bass_kernel_spmd

# Problem constants (hardcoded per contract).
B = 2
N = 4096          # output tokens
K_SUM = 8         # summary tokens
NT = N + K_SUM    # 4104 total tokens
NP = 4224         # padded key count = 33 * 128
C = 192
H = 8
D = 24
NCORES = 8

CI = 512          # query chunk (8 chunks over 4096)
MB = 128          # key block
GROUP = 2         # key blocks per exp group
NCHUNKS = N // CI            # 8
MBLOCKS = NP // MB           # 33
NGROUPS = (MBLOCKS + GROUP - 1) // GROUP  # 17
NSLOTS = (MBLOCKS + 1) // 2  # 17 k-block column slots per strip

LOG2E_128 = 128.0 / float(np.log(2.0))   # fold into Wk on host
ZBIAS = 16256.0                          # 127*128, exact in fp16
# ScalarE true-exp: out = exp(scale*z + bias); the exp(ln 1.040684) factor
# matches the Schraudolph trick's mean multiplicative bias so mixed groups
# stay consistent (softmax cancels the common factor).
ACT_SCALE = float(np.log(2.0)) / 128.0
ACT_BIAS = float(-ZBIAS * np.log(2.0) / 128.0 + np.log(1.040684))

F32 = mybir.dt.float32
F16 = mybir.dt.float16
BF16 = mybir.dt.bfloat16
I16 = mybir.dt.int16

_CACHED = {}


def build_program(num_devices=NCORES):
    nc = bacc.Bacc("TRN2", target_bir_lowering=False, debug=False,
                   num_devices=num_devices)
    xt_d = nc.dram_tensor("XT", [C + 1, NP], F16, kind="ExternalInput")
    wt_d = nc.dram_tensor("WT", [C + 1, 160], F16, kind="ExternalInput")
    wo_d = nc.dram_tensor("WoT", [64, C], F16, kind="ExternalInput")
    out_d = nc.dram_tensor("outT", [N, C], F16, kind="ExternalOutput")

    with tile.TileContext(nc) as tc:
        with tc.tile_pool(name="singles", bufs=1) as singles:
            xt0 = singles.tile([128, NP], F16, tag="xt0")
            xt1 = singles.tile([65, NP], F16, tag="xt1")
            wt0 = singles.tile([128, 160], F16, tag="wt0")
            wt1 = singles.tile([65, 160], F16, tag="wt1")
            wo = singles.tile([64, C], F16, tag="wo")
            qks = singles.tile([96, NP], F16, tag="qks")
            q4 = singles.tile([128, N], F16, tag="q4")
            k4 = singles.tile([128, NSLOTS * MB], F16, tag="k4")
            # Per-head zero-padded V_aug weight tiles: every PV matmul writes
            # the full 64-partition output (tile_position (0,0)) so no
            # col-group-offset matmuls are needed and the first matmul's
            # bank-wide has_written clear covers both heads.
            vaugp = [singles.tile([128, MBLOCKS, 64], BF16, tag=f"vaugp{h}",
                                  name=f"vaugp{h}") for h in range(2)]
            otn = singles.tile([64, N], F16, tag="otn")
            actb = singles.tile([128, 1], F32, tag="actb")
            nc.vector.memset(actb[:], ACT_BIAS)

            # Bias rows for the score matmuls (25th row of each strip):
            # q side streams 1.0, k side holds 16256.0 so PSUM z includes
            # the exponent bias. Padded key columns get z=16256 -> e~1,
            # but their V_aug rows are all zero, contributing nothing.
            # Bias rows live at strip row 24; memsets must start 32-aligned,
            # so fill whole strips and let the q/k scatter DMAs overwrite
            # rows 0:24 (rows 25:31 keep the value but are never streamed).
            for s in range(4):
                nc.vector.memset(q4[32 * s:32 * s + 32, :], 1.0)
                nc.vector.memset(k4[32 * s:32 * s + 32, :], ZBIAS)
            nc.vector.memset(vaugp[0][:, :, :], 0.0)
            nc.vector.memset(vaugp[1][:, :, :], 0.0)

            # Input loads; weights first so the first matmuls aren't queued
            # behind the large XT transfer. XT chunked to start compute early.
            nc.sync.dma_start(out=wt0[:], in_=wt_d[0:128, :])
            nc.sync.dma_start(out=wt1[:], in_=wt_d[128:193, :])
            nc.sync.dma_start(out=wo[:], in_=wo_d[:, :])
            for c0 in range(0, NP, 1056):
                nc.sync.dma_start(out=xt0[:, c0:c0 + 1056],
                                  in_=xt_d[0:128, c0:c0 + 1056])
                nc.gpsimd.dma_start(out=xt1[:, c0:c0 + 1056],
                                    in_=xt_d[128:193, c0:c0 + 1056])

            xts = (xt0, xt1)
            wts = (wt0, wt1)

            # ---- merged qk projection: out rows [q_h0|q_h1|k_h0|k_h1] ----
            with tc.tile_pool(name="qkp", bufs=3, space="PSUM") as qkp:
                for ci in range(9):
                    c0 = ci * CI
                    w = CI if ci < 8 else NP - 8 * CI
                    ps = qkp.tile([96, CI], F32, tag="qk")
                    for kc in range(2):
                        nc.tensor.matmul(
                            ps[:, :w],
                            lhsT=wts[kc][:, 0:96],
                            rhs=xts[kc][:, c0:c0 + w],
                            start=(kc == 0), stop=(kc == 1))
                    if ci % 2 == 0:
                        nc.scalar.copy(out=qks[:, c0:c0 + w], in_=ps[:, :w])
                    else:
                        nc.vector.tensor_copy(out=qks[:, c0:c0 + w],
                                              in_=ps[:, :w])

            # ---- V_aug production: [keys, d] layout ----
            with tc.tile_pool(name="vps", bufs=4, space="PSUM") as vps:
                for mb in range(MBLOCKS):
                    m0 = mb * MB
                    ps = vps.tile([128, 64], F32, tag="v")
                    for kc in range(2):
                        nc.tensor.matmul(
                            ps[:],
                            lhsT=xts[kc][:, m0:m0 + MB],
                            rhs=wts[kc][:, 96:160],
                            start=(kc == 0), stop=(kc == 1))
                    if mb % 2 == 0:
                        nc.scalar.copy(out=vaugp[0][:, mb, 0:32],
                                       in_=ps[:, 0:32])
                        nc.vector.tensor_copy(out=vaugp[1][:, mb, 32:64],
                                              in_=ps[:, 32:64])
                    else:
                        nc.vector.tensor_copy(out=vaugp[0][:, mb, 0:32],
                                              in_=ps[:, 0:32])
                        nc.scalar.copy(out=vaugp[1][:, mb, 32:64],
                                       in_=ps[:, 32:64])

            # ---- scatter q into head strips, k blocks into strip slots ----
            # strip s (partitions 32s..32s+24) serves head s//2; k block j of
            # head h lands on strip 2h+(j&1), column slot j>>1.
            for s in range(4):
                h = s // 2
                for half in range(2):
                    cq = half * (N // 2)
                    eng = nc.sync if (s + half) % 2 == 0 else nc.gpsimd
                    eng.dma_start(
                        out=q4[32 * s:32 * s + 24, cq:cq + N // 2],
                        in_=qks[24 * h:24 * h + 24, cq:cq + N // 2])
            for j in range(MBLOCKS):
                for h in range(2):
                    s = 2 * h + (j & 1)
                    slot = j >> 1
                    eng = nc.sync if (j + h) % 2 == 0 else nc.gpsimd
                    eng.dma_start(
                        out=k4[32 * s:32 * s + 24,
                               slot * MB:(slot + 1) * MB],
                        in_=qks[48 + 24 * h:48 + 24 * h + 24,
                                j * MB:(j + 1) * MB])

            # ---- attention ----
            flats = [(g, h) for g in range(NGROUPS) for h in range(2)]
            with (tc.tile_pool(name="sp", bufs=3, space="PSUM") as sp,
                  tc.tile_pool(name="op", bufs=2, space="PSUM") as op,
                  tc.tile_pool(name="ep", bufs=7) as ep,
                  tc.tile_pool(name="bp", bufs=2) as bp):
                for ci in range(NCHUNKS):
                    c0 = ci * CI
                    o_ps = op.tile([64, CI], F32, tag="o")
                    # 2-flat blocks: 4 S matmuls together (all four row
                    # strips stream concurrently), then both exp ops, then 4
                    # PV matmuls of the block LAG behind, back-to-back, so
                    # the PE stream stays dense and warm.
                    LAG = 2  # blocks
                    pend = {}
                    for b in range(NGROUPS + LAG):
                        if b < NGROUPS:
                            g = b
                            nblk = GROUP if g < NGROUPS - 1 else 1
                            ets = []
                            for h in range(2):
                                s_ps = sp.tile([128, GROUP, CI], F32,
                                               tag="s", name="s_ps")
                                for j in range(nblk):
                                    mb = GROUP * g + j
                                    p0 = 32 * (2 * h + (mb & 1))
                                    slot = mb >> 1
                                    nc.tensor.matmul(
                                        s_ps[:, j, :],
                                        lhsT=k4[p0:p0 + 25,
                                                slot * MB:(slot + 1) * MB],
                                        rhs=q4[p0:p0 + 25, c0:c0 + CI],
                                        start=True, stop=True,
                                        tile_position=(p0, 0),
                                        skip_group_check=True)
                                ets.append((s_ps, h))
                            pend[b] = []
                            for idx, (s_ps, h) in enumerate(ets):
                                e_t = ep.tile([128, GROUP, CI], I16, tag="e",
                                              name="e_t")
                                if (b + h) % 2 == 0:
                                    nc.scalar.activation(
                                        out=e_t[:, 0:nblk, :].bitcast(BF16),
                                        in_=s_ps[:, 0:nblk, :],
                                        func=mybir.ActivationFunctionType.Exp,
                                        bias=actb[:, :], scale=ACT_SCALE)
                                else:
                                    nc.vector.tensor_copy(
                                        out=e_t[:, 0:nblk, :],
                                        in_=s_ps[:, 0:nblk, :])
                                pend[b].append((e_t, h))
                        k_ = b - LAG
                        if 0 <= k_ < NGROUPS:
                            g = k_
                            nblk = GROUP if g < NGROUPS - 1 else 1
                            for e_t, h in pend.pop(k_):
                                for j in range(nblk):
                                    mb = GROUP * g + j
                                    first = (k_ == 0 and h == 0 and j == 0)
                                    nc.tensor.matmul(
                                        o_ps[0:64, :],
                                        lhsT=vaugp[h][:, mb, :],
                                        rhs=e_t[:, j, :].bitcast(BF16),
                                        start=first,
                                        stop=(mb == MBLOCKS - 1),
                                        tile_position=(0, 0),
                                        skip_group_check=True)
                    # normalize: denominator rows are strip cols 0 (h0) and
                    # 32 (h1); engine APs need 32-aligned start partitions,
                    # hence two reciprocal ops into separate base-0 tiles.
                    # reciprocal_approx_fast and partition_broadcast only
                    # work on base-0 APs on hardware; DMA moves h1's
                    # denominator row to partition 0 and its broadcast back
                    # to partitions 32:64 (DMAs handle any partitions).
                    dsb = bp.tile([33, CI], F32, tag="dsb")
                    nc.vector.tensor_copy(out=dsb[0:33, :], in_=o_ps[0:33, :])
                    d2 = bp.tile([1, CI], F32, tag="d2")
                    nc.sync.dma_start(out=d2[0:1, :], in_=dsb[32:33, :])
                    rr0 = bp.tile([1, CI], F32, tag="rr0")
                    rr1 = bp.tile([1, CI], F32, tag="rr1")
                    nc.vector.reciprocal_approx_fast(out=rr0[0:1, :],
                                                     in_=dsb[0:1, :])
                    nc.vector.reciprocal_approx_fast(out=rr1[0:1, :],
                                                     in_=d2[0:1, :])
                    bc = bp.tile([32, CI], F32, tag="bc")
                    bc2 = bp.tile([32, CI], F32, tag="bc2")
                    bc3 = bp.tile([64, CI], F32, tag="bc3")
                    nc.gpsimd.partition_broadcast(bc[0:32, :], rr0[0:1, :])
                    nc.gpsimd.partition_broadcast(bc2[0:32, :], rr1[0:1, :])
                    nc.gpsimd.dma_start(out=bc3[32:64, :], in_=bc2[0:32, :])
                    nc.vector.tensor_mul(out=otn[0:32, c0:c0 + CI],
                                         in0=o_ps[0:32, :], in1=bc[:, :])
                    nc.vector.tensor_mul(out=otn[32:64, c0:c0 + CI],
                                         in0=o_ps[32:64, :],
                                         in1=bc3[32:64, :])

            # ---- output projection (transposed: out[q, c] tiles) ----
            with (tc.tile_pool(name="pp", bufs=4, space="PSUM") as pp,
                  tc.tile_pool(name="ob", bufs=4) as ob):
                for t in range(N // 128):
                    q0 = t * 128
                    ps = pp.tile([128, C], F32, tag="p")
                    nc.tensor.matmul(ps[:], lhsT=otn[:, q0:q0 + 128],
                                     rhs=wo[:, :], start=True, stop=True)
                    so = ob.tile([128, C], F16, tag="so")
                    if t % 2 == 0:
                        nc.scalar.copy(out=so[:], in_=ps[:])
                    else:
                        nc.vector.tensor_copy(out=so[:], in_=ps[:])
                    eng = nc.sync if t % 2 == 0 else nc.gpsimd
                    eng.dma_start(out=out_d[q0:q0 + 128, :], in_=so[:])

    nc.compile()
    return nc


def make_in_maps(X_flat, S_tokens, Wqkv, Wout, temperature):
    temp = np.asarray(temperature, dtype=np.float32).reshape(H)
    Wq = np.asarray(Wqkv[0:C], dtype=np.float32)
    Wk = np.asarray(Wqkv[C:2 * C], dtype=np.float32)
    Wv = np.asarray(Wqkv[2 * C:3 * C], dtype=np.float32)
    Wout = np.asarray(Wout, dtype=np.float32)

    xts = []
    for b in range(B):
        x_in = np.concatenate([np.asarray(X_flat[b], dtype=np.float32),
                               np.asarray(S_tokens[b], dtype=np.float32)],
                              axis=0)
        xt = np.zeros((C + 1, NP), dtype=np.float32)
        xt[:C, :NT] = np.ascontiguousarray(x_in.T)
        xt[C, :NT] = 1.0  # indicator -> ones column of V_aug
        xts.append(xt)

    in_maps = []
    for core in range(NCORES):
        b = core // 4
        h0 = 2 * (core % 4)
        h1 = h0 + 1
        wt = np.zeros((C + 1, 160), dtype=np.float32)
        wt[:C, 0:24] = (Wq[h0 * D:(h0 + 1) * D] * temp[h0]).T
        wt[:C, 24:48] = (Wq[h1 * D:(h1 + 1) * D] * temp[h1]).T
        wt[:C, 48:72] = (Wk[h0 * D:(h0 + 1) * D] * LOG2E_128).T
        wt[:C, 72:96] = (Wk[h1 * D:(h1 + 1) * D] * LOG2E_128).T
        # V_aug columns: h0 ones at 0, v at 1:25; h1 ones at 32, v at 33:57
        # (denominator rows land at 32-aligned strip cols 0 and 32).
        wt[C, 96] = 1.0
        wt[:C, 97:121] = Wv[h0 * D:(h0 + 1) * D].T
        wt[C, 128] = 1.0
        wt[:C, 129:153] = Wv[h1 * D:(h1 + 1) * D].T
        wo = np.zeros((64, C), dtype=np.float32)
        wo[1:25] = Wout[:, h0 * D:(h0 + 1) * D].T
        wo[33:57] = Wout[:, h1 * D:(h1 + 1) * D].T
        in_maps.append({
            "XT": np.ascontiguousarray(xts[b]).astype(np.float16),
            "WT": np.ascontiguousarray(wt).astype(np.float16),
            "WoT": np.ascontiguousarray(wo).astype(np.float16),
        })
    return in_maps


def run(in_maps, **kwargs):
    if "nc" not in _CACHED:
        _CACHED["nc"] = build_program()
    return run_bass_kernel_spmd(_CACHED["nc"], in_maps,
                                core_ids=list(range(NCORES)), **kwargs)


def kernel(X_flat, S_tokens, Wqkv, Wout, temperature):
    in_maps = make_in_maps(X_flat, S_tokens, Wqkv, Wout, temperature)
    res = run(in_maps)
    out = np.zeros((B, N, C), dtype=np.float32)
    for core in range(NCORES):
        out[core // 4] += res.results[core]["outT"].astype(np.float32)
    return out


# revision 21
# speedup vs baseline: 1.0620x; 1.0315x over previous
"""MDTA-style dense attention (B=2, N=4096+8 summary tokens, C=192, H=8, D=24)
on 8 Trainium2 NeuronCores.

Sharding: data-parallel over batch B (2) x tensor-parallel over heads
(4 groups of 2 heads) -> 8 cores. Each core computes attention for one batch
and two heads plus its slice of the qkv projection and the output projection
partial sum; partials are summed on the host.

Device algorithm per core (feature-major layouts):
  - One merged qk projection matmul emits q (temperature-folded) and
    k' = (128/ln2)*k for both heads; V_aug in [keys, d] layout via per-block
    matmuls (ones-indicator feature row produces the denominator column and
    zeros for padded keys).
  - Head h streams on PE row-strips {2h, 2h+1}. Each strip carries a 25th
    bias row (q side 1.0, k side 16256.0) so the score matmul lands
    z = (128*log2 e)*S + 16256 in PSUM directly.
  - "exp": alternating per group between ScalarE ACTIVATE(Exp) (true exp,
    scaled by 1.0407 to match the trick's mean) and a DVE fp32->int16 cast
    whose int16 bits are the bf16 encoding of 2^(z/128-127) (Schraudolph);
    softmax normalization cancels the common factor.
  - PV accumulates both heads into one PSUM bank (h0's first matmul covers
    the full 64 columns via a zero-padded weight tile so the bank-wide
    has_written clear stays correct).
  - Denominator rows sit adjacently (strip cols 31/32); one
    reciprocal_approx_fast + two gpsimd partition broadcasts + one fused
    multiply produce normalized head outputs.
  - Output projection is transposed: otn chunks stationary, Wout streamed,
    giving out[q, c] tiles DMAd straight to a [N, C] output.
"""

import numpy as np

import concourse.bass as bass
import concourse.tile as tile
from concourse import bacc, mybir
from concourse.bass_utils import run_bass_kernel_spmd

# Problem constants (hardcoded per contract).
B = 2
N = 4096          # output tokens
K_SUM = 8         # summary tokens
NT = N + K_SUM    # 4104 total tokens
NP = 4224         # padded key count = 33 * 128
C = 192
H = 8
D = 24
NCORES = 8

CI = 512          # query chunk (8 chunks over 4096)
MB = 128          # key block
GROUP = 2         # key blocks per exp group
NCHUNKS = N // CI            # 8
MBLOCKS = NP // MB           # 33
NGROUPS = (MBLOCKS + GROUP - 1) // GROUP  # 17
NSLOTS = (MBLOCKS + 1) // 2  # 17 k-block column slots per strip

LOG2E_128 = 128.0 / float(np.log(2.0))   # fold into Wk on host
ZBIAS = 16256.0                          # 127*128, exact in fp16
# ScalarE true-exp: out = exp(scale*z + bias); the exp(ln 1.040684) factor
# matches the Schraudolph trick's mean multiplicative bias so mixed groups
# stay consistent (softmax cancels the common factor).
ACT_SCALE = float(np.log(2.0)) / 128.0
ACT_BIAS = float(-ZBIAS * np.log(2.0) / 128.0 + np.log(1.040684))

F32 = mybir.dt.float32
F16 = mybir.dt.float16
BF16 = mybir.dt.bfloat16
I16 = mybir.dt.int16

_CACHED = {}


def build_program(num_devices=NCORES):
    nc = bacc.Bacc("TRN2", target_bir_lowering=False, debug=False,
                   num_devices=num_devices)
    xt_d = nc.dram_tensor("XT", [C + 1, NP], F16, kind="ExternalInput")
    wt_d = nc.dram_tensor("WT", [C + 1, 160], F16, kind="ExternalInput")
    wo_d = nc.dram_tensor("WoT", [64, C], F16, kind="ExternalInput")
    out_d = nc.dram_tensor("outT", [N, C], F16, kind="ExternalOutput")

    with tile.TileContext(nc) as tc:
        with tc.tile_pool(name="singles", bufs=1) as singles:
            xt0 = singles.tile([128, NP], F16, tag="xt0")
            xt1 = singles.tile([65, NP], F16, tag="xt1")
            wt0 = singles.tile([128, 160], F16, tag="wt0")
            wt1 = singles.tile([65, 160], F16, tag="wt1")
            wo = singles.tile([64, C], F16, tag="wo")
            qks = singles.tile([96, NP], F16, tag="qks")
            q4 = singles.tile([128, N], F16, tag="q4")
            k4 = singles.tile([128, NSLOTS * MB], F16, tag="k4")
            # Per-head zero-padded V_aug weight tiles: every PV matmul writes
            # the full 64-partition output (tile_position (0,0)) so no
            # col-group-offset matmuls are needed and the first matmul's
            # bank-wide has_written clear covers both heads.
            vaugp = [singles.tile([128, MBLOCKS, 64], BF16, tag=f"vaugp{h}",
                                  name=f"vaugp{h}") for h in range(2)]
            otn = singles.tile([64, N], F16, tag="otn")
            actb = singles.tile([128, 1], F32, tag="actb")
            nc.vector.memset(actb[:], ACT_BIAS)

            # Bias rows for the score matmuls (25th row of each strip):
            # q side streams 1.0, k side holds 16256.0 so PSUM z includes
            # the exponent bias. Padded key columns get z=16256 -> e~1,
            # but their V_aug rows are all zero, contributing nothing.
            # Bias rows live at strip row 24; memsets must start 32-aligned,
            # so fill whole strips and let the q/k scatter DMAs overwrite
            # rows 0:24 (rows 25:31 keep the value but are never streamed).
            nc.vector.memset(q4[:, :], 1.0)
            nc.vector.memset(k4[:, :], ZBIAS)
            nc.vector.memset(vaugp[0][:, :, :], 0.0)
            nc.vector.memset(vaugp[1][:, :, :], 0.0)

            # Input loads; weights first so the first matmuls aren't queued
            # behind the large XT transfer. XT chunked to start compute early.
            nc.sync.dma_start(out=wt0[:], in_=wt_d[0:128, :])
            nc.sync.dma_start(out=wt1[:], in_=wt_d[128:193, :])
            nc.sync.dma_start(out=wo[:], in_=wo_d[:, :])
            for c0 in range(0, NP, 1056):
                nc.sync.dma_start(out=xt0[:, c0:c0 + 1056],
                                  in_=xt_d[0:128, c0:c0 + 1056])
                nc.gpsimd.dma_start(out=xt1[:, c0:c0 + 1056],
                                    in_=xt_d[128:193, c0:c0 + 1056])

            xts = (xt0, xt1)
            wts = (wt0, wt1)

            # ---- merged qk projection: out rows [q_h0|q_h1|k_h0|k_h1] ----
            with tc.tile_pool(name="qkp", bufs=3, space="PSUM") as qkp:
                for ci in range(9):
                    c0 = ci * CI
                    w = CI if ci < 8 else NP - 8 * CI
                    ps = qkp.tile([96, CI], F32, tag="qk")
                    for kc in range(2):
                        nc.tensor.matmul(
                            ps[:, :w],
                            lhsT=wts[kc][:, 0:96],
                            rhs=xts[kc][:, c0:c0 + w],
                            start=(kc == 0), stop=(kc == 1))
                    if ci % 2 == 0:
                        nc.scalar.copy(out=qks[:, c0:c0 + w], in_=ps[:, :w])
                    else:
                        nc.vector.tensor_copy(out=qks[:, c0:c0 + w],
                                              in_=ps[:, :w])

            # ---- V_aug production: [keys, d] layout ----
            with tc.tile_pool(name="vps", bufs=4, space="PSUM") as vps:
                for mb in range(MBLOCKS):
                    m0 = mb * MB
                    ps = vps.tile([128, 64], F32, tag="v")
                    for kc in range(2):
                        nc.tensor.matmul(
                            ps[:],
                            lhsT=xts[kc][:, m0:m0 + MB],
                            rhs=wts[kc][:, 96:160],
                            start=(kc == 0), stop=(kc == 1))
                    if mb % 2 == 0:
                        nc.scalar.copy(out=vaugp[0][:, mb, 0:32],
                                       in_=ps[:, 0:32])
                        nc.vector.tensor_copy(out=vaugp[1][:, mb, 32:64],
                                              in_=ps[:, 32:64])
                    else:
                        nc.vector.tensor_copy(out=vaugp[0][:, mb, 0:32],
                                              in_=ps[:, 0:32])
                        nc.scalar.copy(out=vaugp[1][:, mb, 32:64],
                                       in_=ps[:, 32:64])

            # ---- scatter q into head strips, k blocks into strip slots ----
            # strip s (partitions 32s..32s+24) serves head s//2; k block j of
            # head h lands on strip 2h+(j&1), column slot j>>1.
            for s in range(4):
                h = s // 2
                for half in range(2):
                    cq = half * (N // 2)
                    eng = nc.sync if (s + half) % 2 == 0 else nc.gpsimd
                    eng.dma_start(
                        out=q4[32 * s:32 * s + 24, cq:cq + N // 2],
                        in_=qks[24 * h:24 * h + 24, cq:cq + N // 2])
            for j in range(MBLOCKS):
                for h in range(2):
                    s = 2 * h + (j & 1)
                    slot = j >> 1
                    eng = nc.sync if (j + h) % 2 == 0 else nc.gpsimd
                    eng.dma_start(
                        out=k4[32 * s:32 * s + 24,
                               slot * MB:(slot + 1) * MB],
                        in_=qks[48 + 24 * h:48 + 24 * h + 24,
                                j * MB:(j + 1) * MB])

            # ---- attention ----
            flats = [(g, h) for g in range(NGROUPS) for h in range(2)]
            with (tc.tile_pool(name="sp", bufs=3, space="PSUM") as sp,
                  tc.tile_pool(name="op", bufs=2, space="PSUM") as op,
                  tc.tile_pool(name="ep", bufs=10) as ep,
                  tc.tile_pool(name="bp", bufs=2) as bp):
                for ci in range(NCHUNKS):
                    c0 = ci * CI
                    o_ps = op.tile([64, CI], F32, tag="o")
                    # 2-flat blocks: 4 S matmuls together (all four row
                    # strips stream concurrently), then both exp ops, then 4
                    # PV matmuls of the block LAG behind, back-to-back, so
                    # the PE stream stays dense and warm.
                    LAG = 3  # blocks
                    pend = {}
                    for b in range(NGROUPS + LAG):
                        if b < NGROUPS:
                            g = b
                            nblk = GROUP if g < NGROUPS - 1 else 1
                            ets = []
                            for h in range(2):
                                s_ps = sp.tile([128, GROUP, CI], F32,
                                               tag="s", name="s_ps")
                                for j in range(nblk):
                                    mb = GROUP * g + j
                                    p0 = 32 * (2 * h + (mb & 1))
                                    slot = mb >> 1
                                    nc.tensor.matmul(
                                        s_ps[:, j, :],
                                        lhsT=k4[p0:p0 + 25,
                                                slot * MB:(slot + 1) * MB],
                                        rhs=q4[p0:p0 + 25, c0:c0 + CI],
                                        start=True, stop=True,
                                        tile_position=(p0, 0),
                                        skip_group_check=True)
                                ets.append((s_ps, h))
                            pend[b] = []
                            for idx, (s_ps, h) in enumerate(ets):
                                e_t = ep.tile([128, GROUP, CI], I16, tag="e",
                                              name="e_t")
                                if (b + h) % 2 == 0 or b == 7:
                                    nc.scalar.activation(
                                        out=e_t[:, 0:nblk, :].bitcast(BF16),
                                        in_=s_ps[:, 0:nblk, :],
                                        func=mybir.ActivationFunctionType.Exp,
                                        bias=actb[:, :], scale=ACT_SCALE)
                                else:
                                    nc.vector.tensor_copy(
                                        out=e_t[:, 0:nblk, :],
                                        in_=s_ps[:, 0:nblk, :])
                                pend[b].append((e_t, h))
                        k_ = b - LAG
                        if 0 <= k_ < NGROUPS:
                            g = k_
                            nblk = GROUP if g < NGROUPS - 1 else 1
                            for e_t, h in pend.pop(k_):
                                for j in range(nblk):
                                    mb = GROUP * g + j
                                    first = (k_ == 0 and h == 0 and j == 0)
                                    nc.tensor.matmul(
                                        o_ps[0:64, :],
                                        lhsT=vaugp[h][:, mb, :],
                                        rhs=e_t[:, j, :].bitcast(BF16),
                                        start=first,
                                        stop=(mb == MBLOCKS - 1),
                                        tile_position=(0, 0),
                                        skip_group_check=True)
                    # normalize: denominator rows are strip cols 0 (h0) and
                    # 32 (h1); engine APs need 32-aligned start partitions,
                    # hence two reciprocal ops into separate base-0 tiles.
                    # reciprocal_approx_fast and partition_broadcast only
                    # work on base-0 APs on hardware; DMA moves h1's
                    # denominator row to partition 0 and its broadcast back
                    # to partitions 32:64 (DMAs handle any partitions).
                    dsb = bp.tile([33, CI], F32, tag="dsb")
                    nc.vector.tensor_copy(out=dsb[0:33, :], in_=o_ps[0:33, :])
                    d2 = bp.tile([1, CI], F32, tag="d2")
                    nc.sync.dma_start(out=d2[0:1, :], in_=dsb[32:33, :])
                    rr0 = bp.tile([1, CI], F32, tag="rr0")
                    rr1 = bp.tile([1, CI], F32, tag="rr1")
                    nc.vector.reciprocal_approx_fast(out=rr0[0:1, :],
                                                     in_=dsb[0:1, :])
                    nc.vector.reciprocal_approx_fast(out=rr1[0:1, :],
                                                     in_=d2[0:1, :])
                    bc = bp.tile([32, CI], F32, tag="bc")
                    bc2 = bp.tile([32, CI], F32, tag="bc2")
                    bc3 = bp.tile([64, CI], F32, tag="bc3")
                    nc.gpsimd.partition_broadcast(bc[0:32, :], rr0[0:1, :])
                    nc.gpsimd.partition_broadcast(bc2[0:32, :], rr1[0:1, :])
                    nc.gpsimd.dma_start(out=bc3[32:64, :], in_=bc2[0:32, :])
                    nc.vector.tensor_mul(out=otn[0:32, c0:c0 + CI],
                                         in0=o_ps[0:32, :], in1=bc[:, :])
                    nc.vector.tensor_mul(out=otn[32:64, c0:c0 + CI],
                                         in0=o_ps[32:64, :],
                                         in1=bc3[32:64, :])

            # ---- output projection (transposed: out[q, c] tiles) ----
            with (tc.tile_pool(name="pp", bufs=4, space="PSUM") as pp,
                  tc.tile_pool(name="ob", bufs=4) as ob):
                for t in range(N // 128):
                    q0 = t * 128
                    ps = pp.tile([128, C], F32, tag="p")
                    nc.tensor.matmul(ps[:], lhsT=otn[:, q0:q0 + 128],
                                     rhs=wo[:, :], start=True, stop=True)
                    so = ob.tile([128, C], F16, tag="so")
                    if t % 2 == 0:
                        nc.scalar.copy(out=so[:], in_=ps[:])
                    else:
                        nc.vector.tensor_copy(out=so[:], in_=ps[:])
                    eng = nc.sync if t % 2 == 0 else nc.gpsimd
                    eng.dma_start(out=out_d[q0:q0 + 128, :], in_=so[:])

    nc.compile()
    return nc


def make_in_maps(X_flat, S_tokens, Wqkv, Wout, temperature):
    temp = np.asarray(temperature, dtype=np.float32).reshape(H)
    Wq = np.asarray(Wqkv[0:C], dtype=np.float32)
    Wk = np.asarray(Wqkv[C:2 * C], dtype=np.float32)
    Wv = np.asarray(Wqkv[2 * C:3 * C], dtype=np.float32)
    Wout = np.asarray(Wout, dtype=np.float32)

    xts = []
    for b in range(B):
        x_in = np.concatenate([np.asarray(X_flat[b], dtype=np.float32),
                               np.asarray(S_tokens[b], dtype=np.float32)],
                              axis=0)
        xt = np.zeros((C + 1, NP), dtype=np.float32)
        xt[:C, :NT] = np.ascontiguousarray(x_in.T)
        xt[C, :NT] = 1.0  # indicator -> ones column of V_aug
        xts.append(xt)

    in_maps = []
    for core in range(NCORES):
        b = core // 4
        h0 = 2 * (core % 4)
        h1 = h0 + 1
        wt = np.zeros((C + 1, 160), dtype=np.float32)
        wt[:C, 0:24] = (Wq[h0 * D:(h0 + 1) * D] * temp[h0]).T
        wt[:C, 24:48] = (Wq[h1 * D:(h1 + 1) * D] * temp[h1]).T
        wt[:C, 48:72] = (Wk[h0 * D:(h0 + 1) * D] * LOG2E_128).T
        wt[:C, 72:96] = (Wk[h1 * D:(h1 + 1) * D] * LOG2E_128).T
        # V_aug columns: h0 ones at 0, v at 1:25; h1 ones at 32, v at 33:57
        # (denominator rows land at 32-aligned strip cols 0 and 32).
        wt[C, 96] = 1.0
        wt[:C, 97:121] = Wv[h0 * D:(h0 + 1) * D].T
        wt[C, 128] = 1.0
        wt[:C, 129:153] = Wv[h1 * D:(h1 + 1) * D].T
        wo = np.zeros((64, C), dtype=np.float32)
        wo[1:25] = Wout[:, h0 * D:(h0 + 1) * D].T
        wo[33:57] = Wout[:, h1 * D:(h1 + 1) * D].T
        in_maps.append({
            "XT": np.ascontiguousarray(xts[b]).astype(np.float16),
            "WT": np.ascontiguousarray(wt).astype(np.float16),
            "WoT": np.ascontiguousarray(wo).astype(np.float16),
        })
    return in_maps


def run(in_maps, **kwargs):
    if "nc" not in _CACHED:
        _CACHED["nc"] = build_program()
    return run_bass_kernel_spmd(_CACHED["nc"], in_maps,
                                core_ids=list(range(NCORES)), **kwargs)


def kernel(X_flat, S_tokens, Wqkv, Wout, temperature):
    in_maps = make_in_maps(X_flat, S_tokens, Wqkv, Wout, temperature)
    res = run(in_maps)
    out = np.zeros((B, N, C), dtype=np.float32)
    for core in range(NCORES):
        out[core // 4] += res.results[core]["outT"].astype(np.float32)
    return out
